# revision 24
# baseline (speedup 1.0000x reference)
"""TRN2 Bass/Tile kernel: BatchNorm1d + 4-head self-attention + out-projection.

Reference computation (b=4, c=256, n=4096, heads=4, d=64):
    xn   = BN(x)  (training-mode stats over batch+length)
    qkv  = w_qkv @ xn ;  q,k,v  (q scaled by d^-0.5)
    out  = softmax(q^T k) @ v^T  per (batch, head)
    y    = w_out @ out + b_out

Sharding over 8 NeuronCores: core i handles (batch i//2, query-half i%2).
Keys/values are processed in the core-local order [mine, other] (softmax and
attention are invariant to key permutation).

Design notes (v2 rewrite over the AllReduce-free baseline):
  - NO cross-core collective: every core receives the other 3 batches
    (fp8 for stats only) and computes the EXACT global BN statistics
    locally (own batch via DVE bn_stats, part of the rest via ACT
    Copy/Square accum_out sums).
  - BN scale folded into the QKV weights; shift becomes per-channel
    biases.  k-projection bias dropped (per-query score shift, softmax
    cancels it).  v-projection bias folded into the OUTPUT bias:
    W_out @ (W_v @ shift) is a per-output-channel constant, computed
    with tiny PE matmuls at startup -> zero steady-state cost.
  - HEAD-PAIR layout: q/k stored [128, hp, n] with head 2hp on
    partitions 0:64 and head 2hp+1 on 64:128 (no zero padding).  The
    scores for both heads of a pair run as TWO CONCURRENT K=64
    row-tiled matmuls (tile_position (0,0)/(64,0)) - measured 1.88x
    the serial K=128 rate on HW.
  - exp split 50/50: even key-chunks on ACT (table exp), odd chunks on
    the DVE as a Schraudolph bf16 bit-trick (fused mult+add to int16).
  - AV: lhsT = vT-block [128key, 65] bf16 (64 v channels + ones column
    -> softmax denominator for free); attn kept per-head at partitions
    0:64 (attn2 [64, h, n]); out-projection runs as 4 accumulating
    K=64 matmuls per 128-channel block (streaming time only depends on
    the moving size, so this costs ~nothing vs K=128).
  - Normalization (deferred into the next pair, hook chunks 2..13):
    ACT/DVE copy the AV PSUM to SBUF + denominator rows to a
    partition-0 tile (PSUM->partition-0 tensor_copy is the only legal
    cross-partition move), DVE reciprocal_approx_fast, gpsimd
    partition_broadcast, one DVE multiply for both heads.
  - Per chunk the AV matmuls run TWO chunks behind the scores so the
    exp engines (strict ACT/DVE alternation, ACT 17 / DVE 15 chunks)
    are never on the PE's critical path; measured steady chunk period
    873 ns (3x512-col streams + ~200ns exposed LDWEIGHTS - walrus
    emits one LDW per matmul, no dedup, '--enable-ldw-opt=false').
  - PSUM: spool 3x[128,2,512] (scores pairs, outproj bursts, startup
    QKV + keep-warm) + avpool 2x[128,512] (AV accumulators) = 8 banks.
  - BN stats: x_rest arrives as 6 fp8 slabs of 2048 cols on TWO DMA
    rings (sync + gpsimd queues, slab0/slab1 ahead of x in ring
    order); ACT reduces slabs 0,2,3 (Copy/Square accum passes), DVE
    bn_stats slab 1, the own batch in halves, then slabs 4,5.
  - k/v projections are fused INTO the first attention pair (produced
    just-in-time, one key-chunk ahead), so ACT/DVE exp work starts
    ~10us earlier; a data-gated burst of full-array keep-warm matmuls
    (1x1 dummies do NOT register on the HAM activity monitor) runs
    during the stats-combine so QKV hits the 2.4GHz clock; the ACT
    sqrt/exp table sets are preloaded off the critical path.
"""

import numpy as np

import concourse.bacc as bacc
import concourse.tile as tile
from concourse import mybir
from concourse.bass_utils import run_bass_kernel_spmd

B, C, N = 4, 256, 4096
H, D = 4, 64
P = 128
CT = C // P            # 2 channel tiles of 128
RB = 2                 # row blocks for q/k rows (256 = 2*128)
HP = 2                 # head pairs
NH = N // 2            # 2048 queries per core
QS = 512               # query subtile (1 PSUM bank of fp32)
NQS = NH // QS         # 4
KC = 128               # key chunk (matmul stationary width)
NKC = N // KC          # 32
EPS = 1e-5
SCALE = D ** -0.5
F32 = mybir.dt.float32
BF16 = mybir.dt.bfloat16
XDT = BF16
F8 = mybir.dt.float8e4
NCORES = 8
WARM = 10          # keep-warm PE matmuls issued right before stats-combine
SCH_A = 184.6650244    # 2^7 / ln 2
SCH_B = 16250.65       # 127*128 - c_opt (half-way rounding compensation)


def _body(tc, x_mine, x_other, x_rest, w_qkvT, w_oT2, bn_w, bn_b, b_out, out):
    from contextlib import ExitStack

    nc = tc.nc
    AF = mybir.ActivationFunctionType
    OP = mybir.AluOpType

    with ExitStack() as ctx:
        big = ctx.enter_context(tc.tile_pool(name="big", bufs=1))
        small = ctx.enter_context(tc.tile_pool(name="small", bufs=1))
        epool = ctx.enter_context(tc.tile_pool(name="epool", bufs=4))
        oupool = ctx.enter_context(tc.tile_pool(name="oupool", bufs=2))
        rpool = ctx.enter_context(tc.tile_pool(name="rpool", bufs=2))
        opool = ctx.enter_context(tc.tile_pool(name="opool", bufs=2))
        spool = ctx.enter_context(tc.tile_pool(name="spool", bufs=3, space="PSUM"))
        avpool = ctx.enter_context(tc.tile_pool(name="avpool", bufs=2, space="PSUM"))

        # ---- loads (two DMA rings: sync + gpsimd, stats slabs first) ----
        xn_sb = big.tile([P, CT, N], XDT, tag="xn")  # RAW x, key order [mine|other]
        xm_r = x_mine.rearrange("(ct p) n -> p ct n", p=P)
        stg = ctx.enter_context(tc.tile_pool(name="stg", bufs=6))
        NRC_ = 2
        RCW_ = N // NRC_
        sts = [stg.tile([P, CT, RCW_], F8, tag="stg", name=f"st{i}")
               for i in range(6)]
        # sync ring: slab0, x_mine, slab2, slab4, slab5, small vecs
        nc.sync.dma_start(out=sts[0], in_=x_rest[0, 0])
        for ct in range(CT):
            for half in range(2):
                nc.sync.dma_start(
                    out=xn_sb[:, ct, half * (NH // 2) : (half + 1) * (NH // 2)],
                    in_=xm_r[:, ct, half * (NH // 2) : (half + 1) * (NH // 2)],
                )
        nc.sync.dma_start(out=sts[2], in_=x_rest[1, 0])
        nc.sync.dma_start(out=sts[4], in_=x_rest[2, 0])
        nc.sync.dma_start(out=sts[5], in_=x_rest[2, 1])
        bnw_sb = small.tile([P, CT, 1], F32)
        nc.sync.dma_start(out=bnw_sb, in_=bn_w)
        bnb_sb = small.tile([P, CT, 1], F32)
        nc.sync.dma_start(out=bnb_sb, in_=bn_b)
        bo_sb = small.tile([P, RB, 1], F32)
        nc.sync.dma_start(out=bo_sb, in_=b_out)
        # gpsimd ring: slab1, slab3, x_other, weights
        nc.gpsimd.dma_start(out=sts[1], in_=x_rest[0, 1])
        nc.gpsimd.dma_start(out=sts[3], in_=x_rest[1, 1])
        nc.gpsimd.dma_start(
            out=xn_sb[:, :, NH:N], in_=x_other.rearrange("(ct p) n -> p ct n", p=P)
        )
        wq_sb = big.tile([P, CT, 3 * C], XDT)
        nc.gpsimd.dma_start(
            out=wq_sb, in_=w_qkvT.rearrange("(ct p) o -> p ct o", p=P)
        )
        wo2_sb = big.tile([D, H, C], XDT)   # w_out^T as [d, h, o]
        nc.gpsimd.dma_start(out=wo2_sb, in_=w_oT2)

        # ---- ACT table preloads -----------------------------------------
        # table sets are exclusive; load the sqrt set early (stats' Copy/
        # Square ride along in any set) so the combine's Sqrt does not pay
        # the ~1.3us ACT_TABLE_LOAD on the critical path.
        tiny = small.tile([1, 2], F32)
        nc.scalar.activation(out=tiny[:, 0:1], in_=bnw_sb[0:1, 0, :],
                             func=AF.Sqrt)

        # ---- BN stats: EXACT global stats computed locally --------------
        # ACT: Sum/Sum^2 accum passes over slabs 0,2,3 (earliest arrivals);
        # DVE: bn_stats over slab1 (arrives before x_mine completes), the
        # own batch in halves (mine half first), then slabs 4,5.
        RCW = RCW_
        SG = N // 512      # own-batch 512-col stat groups per ct
        ACT_SL = (0, 3, 2)
        DVE_SL = (1, 4, 5)
        NACT = len(ACT_SL)
        RG = RCW // 512
        NDVE_R = len(DVE_SL) * RG
        NREC = SG + NDVE_R         # bn_stats records per ct
        NS = N + NDVE_R * 512      # samples covered by bn_stats records
        NT = B * N                 # total samples per channel
        stat6 = small.tile([P, CT, NREC, 6], F32)
        trash = small.tile([P, RCW], BF16)
        acc_x = small.tile([P, CT, NACT], F32)
        acc_x2 = small.tile([P, CT, NACT], F32)

        def dve_slab(si, ri):
            st = sts[si]
            for ct in range(CT):
                xr = st[:, ct, :].rearrange("p (s f) -> p s f", f=512)
                for g in range(RG):
                    nc.vector.bn_stats(
                        out=stat6[:, ct, SG + ri * RG + g, :], in_=xr[:, g, :]
                    )

        def own_half(half):
            for ct in range(CT):
                xm = xn_sb[:, ct, :].rearrange("p (s f) -> p s f", f=512)
                for g in range(half * 4, half * 4 + 4):
                    nc.vector.bn_stats(out=stat6[:, ct, g, :], in_=xm[:, g, :])

        dve_slab(DVE_SL[0], 0)
        own_half(0)
        own_half(1)
        dve_slab(DVE_SL[1], 1)
        dve_slab(DVE_SL[2], 2)
        for ci, si in enumerate(ACT_SL):
            for ct in range(CT):
                nc.scalar.activation(
                    out=trash, in_=sts[si][:, ct, :], func=AF.Copy,
                    accum_out=acc_x[:, ct, ci : ci + 1],
                )
                nc.scalar.activation(
                    out=trash, in_=sts[si][:, ct, :], func=AF.Square,
                    accum_out=acc_x2[:, ct, ci : ci + 1],
                )

        # ---- PE keep-warm ------------------------------------------------
        # The HAM activity monitor runs the PE at half clock until it has
        # seen a ~3.4us busy window.  A dense burst of N=512 matmuls DATA-
        # GATED on stat6 (so the scheduler cannot hoist it) runs during the
        # stats-combine and warms the clock just in time for the QKV
        # projections.
        dum2 = small.tile([P, QS], BF16)
        nc.vector.memset(dum2, 0.001)
        nc.vector.tensor_copy(
            out=dum2[0:1, 0 : NREC * 6],
            in_=stat6[0:1, 0, :, :].rearrange("p a b -> p (a b)"),
        )
        for i in range(WARM):
            scrap = spool.tile([P, 2, QS], F32, tag="sp", name="scrap")
            nc.tensor.matmul(
                out=scrap[:, 0, :], lhsT=dum2[:, 0:P], rhs=dum2,
                start=True, stop=True,
            )

        mv = small.tile([P, CT, 2], F32)
        for ct in range(CT):
            nc.vector.bn_aggr(out=mv[:, ct, :], in_=stat6[:, ct])
        # combine: totals = bn_aggr subset (NS samples) + ACT sums
        sum_t = small.tile([P, CT, 1], F32)
        nc.vector.tensor_reduce(
            out=sum_t, in_=acc_x, axis=mybir.AxisListType.X,
            op=mybir.AluOpType.add,
        )
        sq_t = small.tile([P, CT, 1], F32)
        nc.vector.tensor_reduce(
            out=sq_t, in_=acc_x2, axis=mybir.AxisListType.X,
            op=mybir.AluOpType.add,
        )
        msq_s = small.tile([P, CT, 1], F32)
        nc.vector.tensor_mul(out=msq_s, in0=mv[:, :, 0:1], in1=mv[:, :, 0:1])
        e2_s = small.tile([P, CT, 1], F32)
        nc.vector.tensor_add(out=e2_s, in0=mv[:, :, 1:2], in1=msq_s)
        # sum_t += mean_s * NS ; sq_t += e2_s * NS
        tmp_s = small.tile([P, CT, 1], F32)
        nc.vector.tensor_scalar_mul(out=tmp_s, in0=mv[:, :, 0:1], scalar1=float(NS))
        nc.vector.tensor_add(out=sum_t, in0=sum_t, in1=tmp_s)
        nc.vector.tensor_scalar_mul(out=tmp_s, in0=e2_s, scalar1=float(NS))
        nc.vector.tensor_add(out=sq_t, in0=sq_t, in1=tmp_s)
        mvg = small.tile([P, CT, 2], F32)
        nc.vector.tensor_scalar_mul(
            out=mvg[:, :, 0:1], in0=sum_t, scalar1=1.0 / NT
        )
        nc.vector.tensor_scalar_mul(out=tmp_s, in0=sq_t, scalar1=1.0 / NT)
        nc.vector.tensor_mul(
            out=mvg[:, :, 1:2], in0=mvg[:, :, 0:1], in1=mvg[:, :, 0:1]
        )
        nc.vector.tensor_sub(out=mvg[:, :, 1:2], in0=tmp_s, in1=mvg[:, :, 1:2])
        mv = mvg

        eps_sb = small.tile([P, 1], F32)
        nc.vector.memset(eps_sb, EPS)

        # ---- global mean/var -> s = bn_w * rstd, shift = bn_b - mean*s --
        mean_g = mv[:, :, 0:1]
        var_g = mv[:, :, 1:2]
        sd = small.tile([P, CT, 1], F32)
        nc.scalar.activation(out=sd, in_=var_g, func=AF.Sqrt, bias=eps_sb)
        nc.scalar.activation(out=tiny[:, 1:2], in_=sd[0:1, 0, :], func=AF.Exp)
        rstd = small.tile([P, CT, 1], F32)
        nc.vector.reciprocal(out=rstd, in_=sd)
        s_sb = small.tile([P, CT, 1], F32)
        nc.vector.tensor_mul(out=s_sb, in0=bnw_sb, in1=rstd)
        shift_sb = small.tile([P, CT, 1], F32)
        nc.vector.tensor_mul(out=shift_sb, in0=mean_g, in1=s_sb)
        nc.vector.tensor_sub(out=shift_sb, in0=bnb_sb, in1=shift_sb)
        shift_bf = small.tile([P, CT, 1], BF16)
        nc.vector.tensor_copy(out=shift_bf, in_=shift_sb)

        # ---- biases from the ORIGINAL weights ---------------------------
        # q bias qkb[:, rb] = W_q[rb] @ shift (k bias dropped: softmax
        # cancels a per-query score shift).
        # v bias per head as a [64,1] column: vb4[:, h] = W_v,h @ shift.
        # Output-bias correction: bo_eff = b_out + W_out @ vb  (the v bias
        # contributes attn-weight-sum * vb = vb after normalization).
        bias_ps = spool.tile([P, 2, QS], F32, tag="sp", name="bias")
        for rb in range(RB):
            for ct in range(CT):
                nc.tensor.matmul(
                    out=bias_ps[:, 0, rb : rb + 1],
                    lhsT=wq_sb[:, ct, rb * P : (rb + 1) * P],
                    rhs=shift_bf[:, ct],
                    start=(ct == 0),
                    stop=(ct == CT - 1),
                )
        for h in range(H):
            for ct in range(CT):
                nc.tensor.matmul(
                    out=bias_ps[0:D, 0, RB + h : RB + h + 1],
                    lhsT=wq_sb[:, ct, 2 * C + h * D : 2 * C + (h + 1) * D],
                    rhs=shift_bf[:, ct],
                    start=(ct == 0),
                    stop=(ct == CT - 1),
                )
        qkb_sb = small.tile([P, RB], F32)
        nc.vector.tensor_copy(out=qkb_sb, in_=bias_ps[:, 0, 0:RB])
        vb4 = small.tile([D, H], BF16)
        nc.vector.tensor_copy(out=vb4, in_=bias_ps[0:D, 0, RB : RB + H])
        # bo_eff = b_out + sum_h W_out[:, (h,:)] @ vb4[:, h]
        for rb in range(RB):
            for h in range(H):
                nc.tensor.matmul(
                    out=bias_ps[:, 1, rb : rb + 1],
                    lhsT=wo2_sb[:, h, rb * P : (rb + 1) * P],
                    rhs=vb4[:, h : h + 1],
                    start=(h == 0),
                    stop=(h == H - 1),
                )
        bo_eff = small.tile([P, RB], F32)
        nc.vector.tensor_add(
            out=bo_eff, in0=bias_ps[:, 1, 0:RB],
            in1=bo_sb.rearrange("p rb one -> p (rb one)"),
        )

        # fold diag(s) into the weights, q columns first so q matmuls can
        # start while k/v columns are still being scaled (d^-0.5 is folded
        # into the q columns host-side)
        for sec in range(3):
            for ct in range(CT):
                nc.vector.tensor_scalar_mul(
                    out=wq_sb[:, ct, sec * C : (sec + 1) * C],
                    in0=wq_sb[:, ct, sec * C : (sec + 1) * C],
                    scalar1=s_sb[:, ct],
                )

        # ---- attention state --------------------------------------------
        q2 = big.tile([P, HP, NH], BF16)
        k2 = big.tile([P, HP, N], BF16)
        vT = big.tile([P, NKC, H, D + 1], BF16)
        nc.gpsimd.memset(vT[:, :, :, D : D + 1], 1.0)
        attn2 = big.tile([D, H, NH], BF16)
        out_r = out.rearrange("(rb p) n -> p rb n", p=P)

        # ---- q projection (consumes RAW x; bias fused into copies) ------
        for j in range(NQS):
            ps = spool.tile([P, 2, QS], F32, tag="sp", name="qproj")
            for hp in range(HP):
                for ct in range(CT):
                    nc.tensor.matmul(
                        out=ps[:, hp, :],
                        lhsT=wq_sb[:, ct, hp * P : (hp + 1) * P],
                        rhs=xn_sb[:, ct, j * QS : (j + 1) * QS],
                        start=(ct == 0),
                        stop=(ct == CT - 1),
                    )
            nc.vector.tensor_scalar_add(
                out=q2[:, 0, j * QS : (j + 1) * QS], in0=ps[:, 0, :],
                scalar1=qkb_sb[:, 0:1],
            )
            nc.scalar.activation(
                out=q2[:, 1, j * QS : (j + 1) * QS], in_=ps[:, 1, :],
                func=AF.Identity, bias=qkb_sb[:, 1:2],
            )

        # ---- k/v production (fused into the first pair's chunk loop) ---
        def emit_kproj(ps_slot, hp, jk, eng):
            for ct in range(CT):
                nc.tensor.matmul(
                    out=ps_slot,
                    lhsT=wq_sb[:, ct, C + hp * P : C + (hp + 1) * P],
                    rhs=xn_sb[:, ct, jk * QS : (jk + 1) * QS],
                    start=(ct == 0),
                    stop=(ct == CT - 1),
                )
            dst = k2[:, hp, jk * QS : (jk + 1) * QS]
            if eng == "act":
                nc.scalar.activation(out=dst, in_=ps_slot, func=AF.Identity)
            else:
                nc.vector.tensor_copy(out=dst, in_=ps_slot)

        def emit_vproj(ps_slot, nb):
            for ct in range(CT):
                nc.tensor.matmul(
                    out=ps_slot[:, 0:C],
                    lhsT=xn_sb[:, ct, nb * KC : (nb + 1) * KC],
                    rhs=wq_sb[:, ct, 2 * C : 3 * C],
                    start=(ct == 0),
                    stop=(ct == CT - 1),
                )
            nc.vector.tensor_copy(
                out=vT[:, nb, :, 0:D],
                in_=ps_slot[:, 0:C].rearrange("p (h d) -> p h d", d=D),
            )

        # ---- attention ---------------------------------------------------
        # Normalize/outproj of pair X are deferred into pair X+1 (hooks at
        # fixed chunk indices) so they never stall the exp engines.  The AV
        # matmuls run one chunk behind the scores (pend_av), carried across
        # pair boundaries.
        pend_av = []          # queue of up to 2 pending AV chunk emissions
        pend_norm = [None]

        def emit_av(p, us=(0, 1)):
            e_, c_, avp_, hp_ = p
            for u in us:
                nc.tensor.matmul(
                    out=avp_[u][0 : D + 1, :],
                    lhsT=vT[:, c_, 2 * hp_ + u, :],
                    rhs=e_[:, u, :],
                    start=(c_ == 0),
                    stop=(c_ == NKC - 1),
                )

        def outproj_mm(ps, j_, step):
            rb, h = divmod(step, H)
            nc.tensor.matmul(
                out=ps[:, rb, :],
                lhsT=wo2_sb[:, h, rb * P : (rb + 1) * P],
                rhs=attn2[:, h, j_ * QS : (j_ + 1) * QS],
                start=(h == 0),
                stop=(h == H - 1),
            )

        def outproj_fin(ps, j_):
            o_t = opool.tile([P, RB, QS], F32, tag="o")
            for rb in range(RB):
                nc.scalar.activation(
                    out=o_t[:, rb, :], in_=ps[:, rb, :],
                    func=AF.Identity, bias=bo_eff[:, rb : rb + 1],
                )
            nc.sync.dma_start(
                out=out_r[:, :, j_ * QS : (j_ + 1) * QS], in_=o_t
            )

        def outproj(j_):
            ps = spool.tile([P, 2, QS], F32, tag="sp", name="oproj")
            for step in range(2 * H):
                outproj_mm(ps, j_, step)
            outproj_fin(ps, j_)

        def sch_exp(e_sb, sp):
            # Schraudolph bf16 exp on the DVE: one fused mult+add into
            # int16, bitcast to bf16 (~1.5% rms err)
            nc.vector.tensor_scalar(
                out=e_sb.bitcast(mybir.dt.int16),
                in0=sp,
                scalar1=SCH_A,
                scalar2=SCH_B,
                op0=OP.mult,
                op1=OP.add,
            )

        def pair(j, hp, fused):
            prev = pend_norm[0]
            ou_ref = [None]
            rbc_ref = [None]
            avp = (
                avpool.tile([P, QS], F32, tag="av", name="avp0"),
                avpool.tile([P, QS], F32, tag="av", name="avp1"),
            )
            vt_ref = [None]
            op_ref = [None]
            for c in range(NKC):
                if fused:
                    jk = c // 4
                    if c % 4 == 0:
                        kt = spool.tile([P, 2, QS], F32, tag="sp", name="kproj")
                        emit_kproj(kt[:, 0, :], 0, jk, "act")
                        emit_kproj(kt[:, 1, :], 1, jk, "dve")
                    if c % 2 == 0:
                        vt_ref[0] = spool.tile(
                            [P, 2, QS], F32, tag="sp", name="vproj"
                        )
                    emit_vproj(vt_ref[0][:, c % 2, :], c)
                sp = spool.tile([P, 2, QS], F32, tag="sp", name="sp")
                nc.tensor.matmul(
                    out=sp[:, 0, :],
                    lhsT=k2[0:D, hp, c * KC : (c + 1) * KC],
                    rhs=q2[0:D, hp, j * QS : (j + 1) * QS],
                    start=True, stop=True,
                )
                nc.tensor.matmul(
                    out=sp[:, 1, :],
                    lhsT=k2[D:P, hp, c * KC : (c + 1) * KC],
                    rhs=q2[D:P, hp, j * QS : (j + 1) * QS],
                    start=True, stop=True,
                )
                e_sb = epool.tile([P, 2, QS], BF16, tag="e")
                if fused:
                    if c % 3 == 2:
                        sch_exp(e_sb, sp)
                    else:
                        nc.scalar.activation(out=e_sb, in_=sp, func=AF.Exp)
                elif c % 2 == 0 and c > 0:
                    sch_exp(e_sb, sp)
                else:
                    nc.scalar.activation(out=e_sb, in_=sp, func=AF.Exp)
                # deferred normalize of the previous pair.  ALL reads of
                # pavp[u] must be emitted at c == 1+u (before the new AV
                # matmul to that PSUM bank is emitted).
                if prev is not None:
                    pj, php, pavp = prev
                    if c == 2:
                        ou = oupool.tile([D, 2, QS], F32, tag="ou")
                        ou_ref[0] = ou
                        den = oupool.tile([1, 2, QS], F32, tag="den")
                        rbc_ref[0] = [den, None]
                        nc.scalar.activation(
                            out=ou[:, 0, :], in_=pavp[0][0:D, :],
                            func=AF.Identity,
                        )
                        nc.vector.tensor_copy(
                            out=den[:, 0, :], in_=pavp[0][D : D + 1, :]
                        )
                    elif c == 3:
                        nc.vector.tensor_copy(
                            out=ou_ref[0][:, 1, :], in_=pavp[1][0:D, :]
                        )
                        nc.vector.tensor_copy(
                            out=rbc_ref[0][0][:, 1, :], in_=pavp[1][D : D + 1, :]
                        )
                    elif c == 5:
                        r0 = oupool.tile([1, 2, QS], F32, tag="r0")
                        nc.vector.reciprocal_approx_fast(
                            out=r0, in_=rbc_ref[0][0]
                        )
                        rbc_ref[0][0] = r0
                    elif c == 7:
                        rbc = rpool.tile([D, 2, QS], F32, tag="rbc")
                        rbc_ref[0][1] = rbc
                        nc.gpsimd.partition_broadcast(
                            rbc.rearrange("p a b -> p (a b)"),
                            rbc_ref[0][0][0:1, :, :].rearrange("p a b -> p (a b)"),
                        )
                    elif c == 10:
                        nc.vector.tensor_tensor(
                            out=attn2[:, 2 * php : 2 * php + 2,
                                      pj * QS : (pj + 1) * QS],
                            in0=ou_ref[0],
                            in1=rbc_ref[0][1],
                            op=OP.mult,
                        )
                    elif c == 13 and php == 1:
                        outproj(pj)
                if len(pend_av) >= 2:
                    emit_av(pend_av.pop(0))
                pend_av.append((e_sb, c, avp, hp))
            pend_norm[0] = (j, hp, avp)

        first = True
        for j in range(NQS):
            for hp in range(HP):
                pair(j, hp, fused=first)
                first = False
        while pend_av:
            emit_av(pend_av.pop(0))

        # flush the last pair's normalize + outproj inline.  A few dummy
        # matmuls keep the HAM clock warm through the serial normalize
        # chain so the final out-projection runs at full rate.
        for i in range(8):
            scrap = spool.tile([P, 2, QS], F32, tag="sp", name="scrap2")
            nc.tensor.matmul(
                out=scrap[:, 0, :], lhsT=dum2[:, 0:P], rhs=dum2,
                start=True, stop=True,
            )
        pj, php, pavp = pend_norm[0]
        ou = oupool.tile([D, 2, QS], F32, tag="ou")
        den = oupool.tile([1, 2, QS], F32, tag="den")
        nc.scalar.activation(out=ou[:, 0, :], in_=pavp[0][0:D, :],
                             func=AF.Identity)
        nc.vector.tensor_copy(out=den[:, 0, :], in_=pavp[0][D : D + 1, :])
        nc.vector.tensor_copy(out=ou[:, 1, :], in_=pavp[1][0:D, :])
        nc.vector.tensor_copy(out=den[:, 1, :], in_=pavp[1][D : D + 1, :])
        r0 = oupool.tile([1, 2, QS], F32, tag="r0")
        nc.vector.reciprocal_approx_fast(out=r0, in_=den)
        rbc = rpool.tile([D, 2, QS], F32, tag="rbc")
        nc.gpsimd.partition_broadcast(
            rbc.rearrange("p a b -> p (a b)"),
            r0[0:1, :, :].rearrange("p a b -> p (a b)"),
        )
        nc.vector.tensor_tensor(
            out=attn2[:, 2 * php : 2 * php + 2, pj * QS : (pj + 1) * QS],
            in0=ou, in1=rbc, op=OP.mult,
        )
        outproj(pj)


def build():
    nc = bacc.Bacc(
        "TRN2", target_bir_lowering=False, debug=False, num_devices=NCORES
    )
    x_mine = nc.dram_tensor("x_mine", [C, NH], XDT, kind="ExternalInput").ap()
    x_other = nc.dram_tensor("x_other", [C, NH], XDT, kind="ExternalInput").ap()
    x_rest = nc.dram_tensor(
        "x_rest", [3, 2, P, CT, N // 2], F8, kind="ExternalInput"
    ).ap()
    w_qkvT = nc.dram_tensor("w_qkvT", [C, 3 * C], XDT, kind="ExternalInput").ap()
    w_oT2 = nc.dram_tensor("w_oT2", [D, H, C], XDT, kind="ExternalInput").ap()
    bn_w = nc.dram_tensor("bn_w", [P, CT, 1], F32, kind="ExternalInput").ap()
    bn_b = nc.dram_tensor("bn_b", [P, CT, 1], F32, kind="ExternalInput").ap()
    b_out = nc.dram_tensor("b_out", [P, RB, 1], F32, kind="ExternalInput").ap()
    out = nc.dram_tensor("out", [C, NH], F32, kind="ExternalOutput").ap()
    with tile.TileContext(nc) as tc:
        _body(tc, x_mine, x_other, x_rest, w_qkvT, w_oT2, bn_w, bn_b, b_out, out)
    nc.compile()
    return nc


_nc_cache = None


def make_in_maps(x, bn_weight, bn_bias, w_qkv, w_out, b_out):
    import ml_dtypes

    x = np.ascontiguousarray(np.asarray(x, dtype=np.float32))
    x_bf = x.astype(ml_dtypes.bfloat16)
    x_f8 = x.astype(ml_dtypes.float8_e4m3fn)
    wqT = np.asarray(w_qkv, dtype=np.float32).T.copy()
    wqT[:, 0:C] *= SCALE  # fold d^-0.5 into the q columns
    wqT = wqT.astype(ml_dtypes.bfloat16)
    # w_out^T reorganized as [d, h, o] for the 4-matmul out-projection
    woT2 = np.ascontiguousarray(
        np.asarray(w_out, dtype=np.float32).T.reshape(H, D, C).transpose(1, 0, 2)
    ).astype(ml_dtypes.bfloat16)

    def vec_layout(v):
        v = np.asarray(v, dtype=np.float32)
        return np.ascontiguousarray(v.reshape(CT, P).T.reshape(P, CT, 1))

    bnw = vec_layout(bn_weight)
    bnb = vec_layout(bn_bias)
    bo = vec_layout(b_out)
    in_maps = []
    # x_rest layout [3, nchunk, P, CT, 1024]: contiguous per DMA chunk so the
    # stats-stream DMAs are pure sequential reads (c = ct*P + p)
    xr_all = x_f8.reshape(B, CT, P, 2, N // 2).transpose(0, 3, 2, 1, 4)
    for core in range(NCORES):
        bi, half = divmod(core, 2)
        mine = np.ascontiguousarray(x_bf[bi][:, half * NH : (half + 1) * NH])
        other = np.ascontiguousarray(x_bf[bi][:, (1 - half) * NH : (2 - half) * NH])
        rest = np.ascontiguousarray(xr_all[[b for b in range(B) if b != bi]])
        in_maps.append(
            {
                "x_mine": mine,
                "x_other": other,
                "x_rest": rest,
                "w_qkvT": wqT,
                "w_oT2": woT2,
                "bn_w": bnw,
                "bn_b": bnb,
                "b_out": bo,
            }
        )
    return in_maps


def assemble(results):
    outp = np.empty((B, C, N), np.float32)
    for core in range(NCORES):
        bi, half = divmod(core, 2)
        outp[bi][:, half * NH : (half + 1) * NH] = results[core]["out"]
    return outp


def kernel(x, bn_weight, bn_bias, w_qkv, w_out, b_out):
    global _nc_cache
    if _nc_cache is None:
        _nc_cache = build()
    in_maps = make_in_maps(x, bn_weight, bn_bias, w_qkv, w_out, b_out)
    res = run_bass_kernel_spmd(_nc_cache, in_maps, list(range(NCORES)))
    return assemble(res.results)


if __name__ == "__main__":
    rng = np.random.default_rng(0)
    x = rng.standard_normal((B, C, N), dtype=np.float32)
    w_qkv = rng.standard_normal((3 * C, C), dtype=np.float32) * C**-0.5
    w_out = rng.standard_normal((C, C), dtype=np.float32) * C**-0.5
    y = kernel(
        x,
        np.ones(C, np.float32),
        np.zeros(C, np.float32),
        w_qkv,
        w_out,
        np.zeros(C, np.float32),
    )
    print(y.shape, np.abs(y).max())


# revision 25
# speedup vs baseline: 1.0032x; 1.0032x over previous
"""TRN2 Bass/Tile kernel: BatchNorm1d + 4-head self-attention + out-projection.

Reference computation (b=4, c=256, n=4096, heads=4, d=64):
    xn   = BN(x)  (training-mode stats over batch+length)
    qkv  = w_qkv @ xn ;  q,k,v  (q scaled by d^-0.5)
    out  = softmax(q^T k) @ v^T  per (batch, head)
    y    = w_out @ out + b_out

Sharding over 8 NeuronCores: core i handles (batch i//2, query-half i%2).
Keys/values are processed in the core-local order [mine, other] (softmax and
attention are invariant to key permutation).

Design notes (v2 rewrite over the AllReduce-free baseline):
  - NO cross-core collective: every core receives the other 3 batches
    (fp8 for stats only) and computes the EXACT global BN statistics
    locally (own batch via DVE bn_stats, part of the rest via ACT
    Copy/Square accum_out sums).
  - BN scale folded into the QKV weights; shift becomes per-channel
    biases.  k-projection bias dropped (per-query score shift, softmax
    cancels it).  v-projection bias folded into the OUTPUT bias:
    W_out @ (W_v @ shift) is a per-output-channel constant, computed
    with tiny PE matmuls at startup -> zero steady-state cost.
  - HEAD-PAIR layout: q/k stored [128, hp, n] with head 2hp on
    partitions 0:64 and head 2hp+1 on 64:128 (no zero padding).  The
    scores for both heads of a pair run as TWO CONCURRENT K=64
    row-tiled matmuls (tile_position (0,0)/(64,0)) - measured 1.88x
    the serial K=128 rate on HW.
  - exp split 50/50: even key-chunks on ACT (table exp), odd chunks on
    the DVE as a Schraudolph bf16 bit-trick (fused mult+add to int16).
  - AV: lhsT = vT-block [128key, 65] bf16 (64 v channels + ones column
    -> softmax denominator for free); attn kept per-head at partitions
    0:64 (attn2 [64, h, n]); out-projection runs as 4 accumulating
    K=64 matmuls per 128-channel block (streaming time only depends on
    the moving size, so this costs ~nothing vs K=128).
  - Normalization (deferred into the next pair, hook chunks 2..13):
    ACT/DVE copy the AV PSUM to SBUF + denominator rows to a
    partition-0 tile (PSUM->partition-0 tensor_copy is the only legal
    cross-partition move), DVE reciprocal_approx_fast, gpsimd
    partition_broadcast, one DVE multiply for both heads.
  - Per chunk the AV matmuls run TWO chunks behind the scores so the
    exp engines (strict ACT/DVE alternation, ACT 17 / DVE 15 chunks)
    are never on the PE's critical path; measured steady chunk period
    873 ns (3x512-col streams + ~200ns exposed LDWEIGHTS - walrus
    emits one LDW per matmul, no dedup, '--enable-ldw-opt=false').
  - PSUM: spool 3x[128,2,512] (scores pairs, outproj bursts, startup
    QKV + keep-warm) + avpool 2x[128,512] (AV accumulators) = 8 banks.
  - BN stats: x_rest arrives as 6 fp8 slabs of 2048 cols on TWO DMA
    rings (sync + gpsimd queues, slab0/slab1 ahead of x in ring
    order); ACT reduces slabs 0,2,3 (Copy/Square accum passes), DVE
    bn_stats slab 1, the own batch in halves, then slabs 4,5.
  - k/v projections are fused INTO the first attention pair (produced
    just-in-time, one key-chunk ahead), so ACT/DVE exp work starts
    ~10us earlier; a data-gated burst of full-array keep-warm matmuls
    (1x1 dummies do NOT register on the HAM activity monitor) runs
    during the stats-combine so QKV hits the 2.4GHz clock; the ACT
    sqrt/exp table sets are preloaded off the critical path.
"""

import numpy as np

import concourse.bacc as bacc
import concourse.tile as tile
from concourse import mybir
from concourse.bass_utils import run_bass_kernel_spmd

B, C, N = 4, 256, 4096
H, D = 4, 64
P = 128
CT = C // P            # 2 channel tiles of 128
RB = 2                 # row blocks for q/k rows (256 = 2*128)
HP = 2                 # head pairs
NH = N // 2            # 2048 queries per core
QS = 512               # query subtile (1 PSUM bank of fp32)
NQS = NH // QS         # 4
KC = 128               # key chunk (matmul stationary width)
NKC = N // KC          # 32
EPS = 1e-5
SCALE = D ** -0.5
F32 = mybir.dt.float32
BF16 = mybir.dt.bfloat16
XDT = BF16
F8 = mybir.dt.float8e4
NCORES = 8
WARM = 10          # keep-warm PE matmuls issued right before stats-combine
SCH_A = 184.6650244    # 2^7 / ln 2
SCH_B = 16250.65       # 127*128 - c_opt (half-way rounding compensation)


def _body(tc, x_mine, x_other, x_rest, w_qkvT, w_oT2, bn_w, bn_b, b_out, out):
    from contextlib import ExitStack

    nc = tc.nc
    AF = mybir.ActivationFunctionType
    OP = mybir.AluOpType

    with ExitStack() as ctx:
        big = ctx.enter_context(tc.tile_pool(name="big", bufs=1))
        small = ctx.enter_context(tc.tile_pool(name="small", bufs=1))
        epool = ctx.enter_context(tc.tile_pool(name="epool", bufs=4))
        oupool = ctx.enter_context(tc.tile_pool(name="oupool", bufs=2))
        rpool = ctx.enter_context(tc.tile_pool(name="rpool", bufs=2))
        opool = ctx.enter_context(tc.tile_pool(name="opool", bufs=2))
        spool = ctx.enter_context(tc.tile_pool(name="spool", bufs=3, space="PSUM"))
        avpool = ctx.enter_context(tc.tile_pool(name="avpool", bufs=2, space="PSUM"))

        # ---- loads (two DMA rings: sync + gpsimd, stats slabs first) ----
        xn_sb = big.tile([P, CT, N], XDT, tag="xn")  # RAW x, key order [mine|other]
        xm_r = x_mine.rearrange("(ct p) n -> p ct n", p=P)
        stg = ctx.enter_context(tc.tile_pool(name="stg", bufs=6))
        NRC_ = 2
        RCW_ = N // NRC_
        sts = [stg.tile([P, CT, RCW_], F8, tag="stg", name=f"st{i}")
               for i in range(6)]
        # sync ring: slab0, x_mine, slab2, slab4, slab5, small vecs
        nc.sync.dma_start(out=sts[0], in_=x_rest[0, 0])
        for ct in range(CT):
            for half in range(2):
                nc.sync.dma_start(
                    out=xn_sb[:, ct, half * (NH // 2) : (half + 1) * (NH // 2)],
                    in_=xm_r[:, ct, half * (NH // 2) : (half + 1) * (NH // 2)],
                )
        nc.sync.dma_start(out=sts[2], in_=x_rest[1, 0])
        nc.sync.dma_start(out=sts[4], in_=x_rest[2, 0])
        nc.sync.dma_start(out=sts[5], in_=x_rest[2, 1])
        bnw_sb = small.tile([P, CT, 1], F32)
        nc.sync.dma_start(out=bnw_sb, in_=bn_w)
        bnb_sb = small.tile([P, CT, 1], F32)
        nc.sync.dma_start(out=bnb_sb, in_=bn_b)
        bo_sb = small.tile([P, RB, 1], F32)
        nc.sync.dma_start(out=bo_sb, in_=b_out)
        # gpsimd ring: slab1, slab3, x_other, weights
        nc.gpsimd.dma_start(out=sts[1], in_=x_rest[0, 1])
        nc.gpsimd.dma_start(out=sts[3], in_=x_rest[1, 1])
        nc.gpsimd.dma_start(
            out=xn_sb[:, :, NH:N], in_=x_other.rearrange("(ct p) n -> p ct n", p=P)
        )
        wq_sb = big.tile([P, CT, 3 * C], XDT)
        nc.gpsimd.dma_start(
            out=wq_sb, in_=w_qkvT.rearrange("(ct p) o -> p ct o", p=P)
        )
        wo2_sb = big.tile([D, H, C], XDT)   # w_out^T as [d, h, o]
        nc.gpsimd.dma_start(out=wo2_sb, in_=w_oT2)

        # ---- ACT table preloads -----------------------------------------
        # table sets are exclusive; load the sqrt set early (stats' Copy/
        # Square ride along in any set) so the combine's Sqrt does not pay
        # the ~1.3us ACT_TABLE_LOAD on the critical path.
        tiny = small.tile([1, 2], F32)
        nc.scalar.activation(out=tiny[:, 0:1], in_=bnw_sb[0:1, 0, :],
                             func=AF.Sqrt)

        # ---- BN stats: EXACT global stats computed locally --------------
        # ACT: Sum/Sum^2 accum passes over slabs 0,2,3 (earliest arrivals);
        # DVE: bn_stats over slab1 (arrives before x_mine completes), the
        # own batch in halves (mine half first), then slabs 4,5.
        RCW = RCW_
        SG = N // 512      # own-batch 512-col stat groups per ct
        ACT_SL = (0, 3, 2)
        DVE_SL = (1, 4, 5)
        NACT = len(ACT_SL)
        RG = RCW // 512
        NDVE_R = len(DVE_SL) * RG
        NREC = SG + NDVE_R         # bn_stats records per ct
        NS = N + NDVE_R * 512      # samples covered by bn_stats records
        NT = B * N                 # total samples per channel
        stat6 = small.tile([P, CT, NREC, 6], F32)
        trash = small.tile([P, RCW], BF16)
        acc_x = small.tile([P, CT, NACT], F32)
        acc_x2 = small.tile([P, CT, NACT], F32)

        def dve_slab(si, ri):
            st = sts[si]
            for ct in range(CT):
                xr = st[:, ct, :].rearrange("p (s f) -> p s f", f=512)
                for g in range(RG):
                    nc.vector.bn_stats(
                        out=stat6[:, ct, SG + ri * RG + g, :], in_=xr[:, g, :]
                    )

        def own_half(half):
            for ct in range(CT):
                xm = xn_sb[:, ct, :].rearrange("p (s f) -> p s f", f=512)
                for g in range(half * 4, half * 4 + 4):
                    nc.vector.bn_stats(out=stat6[:, ct, g, :], in_=xm[:, g, :])

        dve_slab(DVE_SL[0], 0)
        own_half(0)
        own_half(1)
        dve_slab(DVE_SL[1], 1)
        dve_slab(DVE_SL[2], 2)
        for ci, si in enumerate(ACT_SL):
            for ct in range(CT):
                nc.scalar.activation(
                    out=trash, in_=sts[si][:, ct, :], func=AF.Copy,
                    accum_out=acc_x[:, ct, ci : ci + 1],
                )
                nc.scalar.activation(
                    out=trash, in_=sts[si][:, ct, :], func=AF.Square,
                    accum_out=acc_x2[:, ct, ci : ci + 1],
                )

        # ---- PE keep-warm ------------------------------------------------
        # The HAM activity monitor runs the PE at half clock until it has
        # seen a ~3.4us busy window.  A dense burst of N=512 matmuls DATA-
        # GATED on stat6 (so the scheduler cannot hoist it) runs during the
        # stats-combine and warms the clock just in time for the QKV
        # projections.
        dum2 = small.tile([P, QS], BF16)
        nc.vector.memset(dum2, 0.001)
        nc.vector.tensor_copy(
            out=dum2[0:1, 0 : NREC * 6],
            in_=stat6[0:1, 0, :, :].rearrange("p a b -> p (a b)"),
        )
        for i in range(WARM):
            scrap = spool.tile([P, 2, QS], F32, tag="sp", name="scrap")
            nc.tensor.matmul(
                out=scrap[:, 0, :], lhsT=dum2[:, 0:P], rhs=dum2,
                start=True, stop=True,
            )

        mv = small.tile([P, CT, 2], F32)
        for ct in range(CT):
            nc.vector.bn_aggr(out=mv[:, ct, :], in_=stat6[:, ct])
        # combine: totals = bn_aggr subset (NS samples) + ACT sums
        sum_t = small.tile([P, CT, 1], F32)
        nc.vector.tensor_reduce(
            out=sum_t, in_=acc_x, axis=mybir.AxisListType.X,
            op=mybir.AluOpType.add,
        )
        sq_t = small.tile([P, CT, 1], F32)
        nc.vector.tensor_reduce(
            out=sq_t, in_=acc_x2, axis=mybir.AxisListType.X,
            op=mybir.AluOpType.add,
        )
        msq_s = small.tile([P, CT, 1], F32)
        nc.vector.tensor_mul(out=msq_s, in0=mv[:, :, 0:1], in1=mv[:, :, 0:1])
        e2_s = small.tile([P, CT, 1], F32)
        nc.vector.tensor_add(out=e2_s, in0=mv[:, :, 1:2], in1=msq_s)
        # sum_t += mean_s * NS ; sq_t += e2_s * NS
        tmp_s = small.tile([P, CT, 1], F32)
        nc.vector.tensor_scalar_mul(out=tmp_s, in0=mv[:, :, 0:1], scalar1=float(NS))
        nc.vector.tensor_add(out=sum_t, in0=sum_t, in1=tmp_s)
        nc.vector.tensor_scalar_mul(out=tmp_s, in0=e2_s, scalar1=float(NS))
        nc.vector.tensor_add(out=sq_t, in0=sq_t, in1=tmp_s)
        mvg = small.tile([P, CT, 2], F32)
        nc.vector.tensor_scalar_mul(
            out=mvg[:, :, 0:1], in0=sum_t, scalar1=1.0 / NT
        )
        nc.vector.tensor_scalar_mul(out=tmp_s, in0=sq_t, scalar1=1.0 / NT)
        nc.vector.tensor_mul(
            out=mvg[:, :, 1:2], in0=mvg[:, :, 0:1], in1=mvg[:, :, 0:1]
        )
        nc.vector.tensor_sub(out=mvg[:, :, 1:2], in0=tmp_s, in1=mvg[:, :, 1:2])
        mv = mvg

        eps_sb = small.tile([P, 1], F32)
        nc.vector.memset(eps_sb, EPS)

        # ---- global mean/var -> s = bn_w * rstd, shift = bn_b - mean*s --
        mean_g = mv[:, :, 0:1]
        var_g = mv[:, :, 1:2]
        sd = small.tile([P, CT, 1], F32)
        nc.scalar.activation(out=sd, in_=var_g, func=AF.Sqrt, bias=eps_sb)
        nc.scalar.activation(out=tiny[:, 1:2], in_=sd[0:1, 0, :], func=AF.Exp)
        rstd = small.tile([P, CT, 1], F32)
        nc.vector.reciprocal(out=rstd, in_=sd)
        s_sb = small.tile([P, CT, 1], F32)
        nc.vector.tensor_mul(out=s_sb, in0=bnw_sb, in1=rstd)
        shift_sb = small.tile([P, CT, 1], F32)
        nc.vector.tensor_mul(out=shift_sb, in0=mean_g, in1=s_sb)
        nc.vector.tensor_sub(out=shift_sb, in0=bnb_sb, in1=shift_sb)
        shift_bf = small.tile([P, CT, 1], BF16)
        nc.vector.tensor_copy(out=shift_bf, in_=shift_sb)

        # ---- biases from the ORIGINAL weights ---------------------------
        # q bias qkb[:, rb] = W_q[rb] @ shift (k bias dropped: softmax
        # cancels a per-query score shift).
        # v bias per head as a [64,1] column: vb4[:, h] = W_v,h @ shift.
        # Output-bias correction: bo_eff = b_out + W_out @ vb  (the v bias
        # contributes attn-weight-sum * vb = vb after normalization).
        bias_ps = spool.tile([P, 2, QS], F32, tag="sp", name="bias")
        for rb in range(RB):
            for ct in range(CT):
                nc.tensor.matmul(
                    out=bias_ps[:, 0, rb : rb + 1],
                    lhsT=wq_sb[:, ct, rb * P : (rb + 1) * P],
                    rhs=shift_bf[:, ct],
                    start=(ct == 0),
                    stop=(ct == CT - 1),
                )
        for h in range(H):
            for ct in range(CT):
                nc.tensor.matmul(
                    out=bias_ps[0:D, 0, RB + h : RB + h + 1],
                    lhsT=wq_sb[:, ct, 2 * C + h * D : 2 * C + (h + 1) * D],
                    rhs=shift_bf[:, ct],
                    start=(ct == 0),
                    stop=(ct == CT - 1),
                )
        qkb_sb = small.tile([P, RB], F32)
        nc.vector.tensor_copy(out=qkb_sb, in_=bias_ps[:, 0, 0:RB])
        vb4 = small.tile([D, H], BF16)
        nc.vector.tensor_copy(out=vb4, in_=bias_ps[0:D, 0, RB : RB + H])
        # bo_eff = b_out + sum_h W_out[:, (h,:)] @ vb4[:, h]
        for rb in range(RB):
            for h in range(H):
                nc.tensor.matmul(
                    out=bias_ps[:, 1, rb : rb + 1],
                    lhsT=wo2_sb[:, h, rb * P : (rb + 1) * P],
                    rhs=vb4[:, h : h + 1],
                    start=(h == 0),
                    stop=(h == H - 1),
                )
        bo_eff = small.tile([P, RB], F32)
        nc.vector.tensor_add(
            out=bo_eff, in0=bias_ps[:, 1, 0:RB],
            in1=bo_sb.rearrange("p rb one -> p (rb one)"),
        )

        # fold diag(s) into the weights, q columns first so q matmuls can
        # start while k/v columns are still being scaled (d^-0.5 is folded
        # into the q columns host-side)
        for sec in range(3):
            for ct in range(CT):
                nc.vector.tensor_scalar_mul(
                    out=wq_sb[:, ct, sec * C : (sec + 1) * C],
                    in0=wq_sb[:, ct, sec * C : (sec + 1) * C],
                    scalar1=s_sb[:, ct],
                )

        # ---- attention state --------------------------------------------
        q2 = big.tile([P, HP, NH], BF16)
        k2 = big.tile([P, HP, N], BF16)
        vT = big.tile([P, NKC, H, D + 1], BF16)
        nc.gpsimd.memset(vT[:, :, :, D : D + 1], 1.0)
        attn2 = big.tile([D, H, NH], BF16)
        out_r = out.rearrange("(rb p) n -> p rb n", p=P)

        # ---- q projection (consumes RAW x; bias fused into copies) ------
        for j in range(NQS):
            ps = spool.tile([P, 2, QS], F32, tag="sp", name="qproj")
            for hp in range(HP):
                for ct in range(CT):
                    nc.tensor.matmul(
                        out=ps[:, hp, :],
                        lhsT=wq_sb[:, ct, hp * P : (hp + 1) * P],
                        rhs=xn_sb[:, ct, j * QS : (j + 1) * QS],
                        start=(ct == 0),
                        stop=(ct == CT - 1),
                    )
            nc.vector.tensor_scalar_add(
                out=q2[:, 0, j * QS : (j + 1) * QS], in0=ps[:, 0, :],
                scalar1=qkb_sb[:, 0:1],
            )
            nc.scalar.activation(
                out=q2[:, 1, j * QS : (j + 1) * QS], in_=ps[:, 1, :],
                func=AF.Identity, bias=qkb_sb[:, 1:2],
            )

        # ---- k/v production (fused into the first pair's chunk loop) ---
        def emit_kproj(ps_slot, hp, jk, eng):
            for ct in range(CT):
                nc.tensor.matmul(
                    out=ps_slot,
                    lhsT=wq_sb[:, ct, C + hp * P : C + (hp + 1) * P],
                    rhs=xn_sb[:, ct, jk * QS : (jk + 1) * QS],
                    start=(ct == 0),
                    stop=(ct == CT - 1),
                )
            dst = k2[:, hp, jk * QS : (jk + 1) * QS]
            if eng == "act":
                nc.scalar.activation(out=dst, in_=ps_slot, func=AF.Identity)
            else:
                nc.vector.tensor_copy(out=dst, in_=ps_slot)

        def emit_vproj(ps_slot, nb):
            for ct in range(CT):
                nc.tensor.matmul(
                    out=ps_slot[:, 0:C],
                    lhsT=xn_sb[:, ct, nb * KC : (nb + 1) * KC],
                    rhs=wq_sb[:, ct, 2 * C : 3 * C],
                    start=(ct == 0),
                    stop=(ct == CT - 1),
                )
            nc.vector.tensor_copy(
                out=vT[:, nb, :, 0:D],
                in_=ps_slot[:, 0:C].rearrange("p (h d) -> p h d", d=D),
            )

        # ---- attention ---------------------------------------------------
        # Normalize/outproj of pair X are deferred into pair X+1 (hooks at
        # fixed chunk indices) so they never stall the exp engines.  The AV
        # matmuls run one chunk behind the scores (pend_av), carried across
        # pair boundaries.
        pend_av = []          # queue of up to 2 pending AV chunk emissions
        pend_norm = [None]

        def emit_av(p, us=(0, 1)):
            e_, c_, avp_, hp_ = p
            for u in us:
                nc.tensor.matmul(
                    out=avp_[u][0 : D + 1, :],
                    lhsT=vT[:, c_, 2 * hp_ + u, :],
                    rhs=e_[:, u, :],
                    start=(c_ == 0),
                    stop=(c_ == NKC - 1),
                )

        def outproj_mm(ps, j_, step):
            rb, h = divmod(step, H)
            nc.tensor.matmul(
                out=ps[:, rb, :],
                lhsT=wo2_sb[:, h, rb * P : (rb + 1) * P],
                rhs=attn2[:, h, j_ * QS : (j_ + 1) * QS],
                start=(h == 0),
                stop=(h == H - 1),
            )

        def outproj_fin(ps, j_):
            o_t = opool.tile([P, RB, QS], F32, tag="o")
            for rb in range(RB):
                nc.scalar.activation(
                    out=o_t[:, rb, :], in_=ps[:, rb, :],
                    func=AF.Identity, bias=bo_eff[:, rb : rb + 1],
                )
            nc.sync.dma_start(
                out=out_r[:, :, j_ * QS : (j_ + 1) * QS], in_=o_t
            )

        def outproj(j_):
            ps = spool.tile([P, 2, QS], F32, tag="sp", name="oproj")
            for step in range(2 * H):
                outproj_mm(ps, j_, step)
            outproj_fin(ps, j_)

        def sch_exp(e_sb, sp):
            # Schraudolph bf16 exp on the DVE: one fused mult+add into
            # int16, bitcast to bf16 (~1.5% rms err)
            nc.vector.tensor_scalar(
                out=e_sb.bitcast(mybir.dt.int16),
                in0=sp,
                scalar1=SCH_A,
                scalar2=SCH_B,
                op0=OP.mult,
                op1=OP.add,
            )

        def pair(j, hp, fused):
            prev = pend_norm[0]
            ou_ref = [None]
            rbc_ref = [None]
            avp = (
                avpool.tile([P, QS], F32, tag="av", name="avp0"),
                avpool.tile([P, QS], F32, tag="av", name="avp1"),
            )
            vt_ref = [None]
            op_ref = [None]
            for c in range(NKC):
                if fused:
                    jk = c // 4
                    if c % 4 == 0:
                        kt = spool.tile([P, 2, QS], F32, tag="sp", name="kproj")
                        emit_kproj(kt[:, 0, :], 0, jk, "act")
                        emit_kproj(kt[:, 1, :], 1, jk, "dve")
                    if c % 2 == 0:
                        vt_ref[0] = spool.tile(
                            [P, 2, QS], F32, tag="sp", name="vproj"
                        )
                    emit_vproj(vt_ref[0][:, c % 2, :], c)
                sp = spool.tile([P, 2, QS], F32, tag="sp", name="sp")
                nc.tensor.matmul(
                    out=sp[:, 0, :],
                    lhsT=k2[0:D, hp, c * KC : (c + 1) * KC],
                    rhs=q2[0:D, hp, j * QS : (j + 1) * QS],
                    start=True, stop=True,
                )
                nc.tensor.matmul(
                    out=sp[:, 1, :],
                    lhsT=k2[D:P, hp, c * KC : (c + 1) * KC],
                    rhs=q2[D:P, hp, j * QS : (j + 1) * QS],
                    start=True, stop=True,
                )
                e_sb = epool.tile([P, 2, QS], BF16, tag="e")
                if fused:
                    if c % 3 == 2:
                        sch_exp(e_sb, sp)
                    else:
                        nc.scalar.activation(out=e_sb, in_=sp, func=AF.Exp)
                elif c % 2 == 0 and c > 0:
                    sch_exp(e_sb, sp)
                else:
                    nc.scalar.activation(out=e_sb, in_=sp, func=AF.Exp)
                # deferred normalize of the previous pair.  ALL reads of
                # pavp[u] must be emitted at c == 1+u (before the new AV
                # matmul to that PSUM bank is emitted).
                if prev is not None:
                    pj, php, pavp = prev
                    if c == 2:
                        ou = oupool.tile([D, 2, QS], F32, tag="ou")
                        ou_ref[0] = ou
                        den = oupool.tile([1, 2, QS], F32, tag="den")
                        rbc_ref[0] = [den, None]
                        nc.scalar.activation(
                            out=ou[:, 0, :], in_=pavp[0][0:D, :],
                            func=AF.Identity,
                        )
                        nc.vector.tensor_copy(
                            out=den[:, 0, :], in_=pavp[0][D : D + 1, :]
                        )
                    elif c == 3:
                        nc.vector.tensor_copy(
                            out=ou_ref[0][:, 1, :], in_=pavp[1][0:D, :]
                        )
                        nc.vector.tensor_copy(
                            out=rbc_ref[0][0][:, 1, :], in_=pavp[1][D : D + 1, :]
                        )
                    elif c == 5:
                        r0 = oupool.tile([1, 2, QS], F32, tag="r0")
                        nc.vector.reciprocal_approx_fast(
                            out=r0, in_=rbc_ref[0][0]
                        )
                        rbc_ref[0][0] = r0
                    elif c == 7:
                        rbc = rpool.tile([D, 2, QS], F32, tag="rbc")
                        rbc_ref[0][1] = rbc
                        for u in range(2):
                            nc.gpsimd.partition_broadcast(
                                rbc[:, u, :], rbc_ref[0][0][0:1, u, :]
                            )
                    elif c == 10:
                        nc.vector.tensor_tensor(
                            out=attn2[:, 2 * php : 2 * php + 2,
                                      pj * QS : (pj + 1) * QS],
                            in0=ou_ref[0],
                            in1=rbc_ref[0][1],
                            op=OP.mult,
                        )
                    elif c == 13 and php == 1:
                        outproj(pj)
                if len(pend_av) >= 2:
                    emit_av(pend_av.pop(0))
                pend_av.append((e_sb, c, avp, hp))
            pend_norm[0] = (j, hp, avp)

        first = True
        for j in range(NQS):
            for hp in range(HP):
                pair(j, hp, fused=first)
                first = False
        while pend_av:
            emit_av(pend_av.pop(0))

        # flush the last pair's normalize + outproj inline.  A few dummy
        # matmuls keep the HAM clock warm through the serial normalize
        # chain so the final out-projection runs at full rate.
        for i in range(8):
            scrap = spool.tile([P, 2, QS], F32, tag="sp", name="scrap2")
            nc.tensor.matmul(
                out=scrap[:, 0, :], lhsT=dum2[:, 0:P], rhs=dum2,
                start=True, stop=True,
            )
        pj, php, pavp = pend_norm[0]
        ou = oupool.tile([D, 2, QS], F32, tag="ou")
        den = oupool.tile([1, 2, QS], F32, tag="den")
        nc.scalar.activation(out=ou[:, 0, :], in_=pavp[0][0:D, :],
                             func=AF.Identity)
        nc.vector.tensor_copy(out=den[:, 0, :], in_=pavp[0][D : D + 1, :])
        nc.vector.tensor_copy(out=ou[:, 1, :], in_=pavp[1][0:D, :])
        nc.vector.tensor_copy(out=den[:, 1, :], in_=pavp[1][D : D + 1, :])
        r0 = oupool.tile([1, 2, QS], F32, tag="r0")
        nc.vector.reciprocal_approx_fast(out=r0, in_=den)
        rbc = rpool.tile([D, 2, QS], F32, tag="rbc")
        nc.gpsimd.partition_broadcast(
            rbc.rearrange("p a b -> p (a b)"),
            r0[0:1, :, :].rearrange("p a b -> p (a b)"),
        )
        nc.vector.tensor_tensor(
            out=attn2[:, 2 * php : 2 * php + 2, pj * QS : (pj + 1) * QS],
            in0=ou, in1=rbc, op=OP.mult,
        )
        outproj(pj)


def build():
    nc = bacc.Bacc(
        "TRN2", target_bir_lowering=False, debug=False, num_devices=NCORES
    )
    x_mine = nc.dram_tensor("x_mine", [C, NH], XDT, kind="ExternalInput").ap()
    x_other = nc.dram_tensor("x_other", [C, NH], XDT, kind="ExternalInput").ap()
    x_rest = nc.dram_tensor(
        "x_rest", [3, 2, P, CT, N // 2], F8, kind="ExternalInput"
    ).ap()
    w_qkvT = nc.dram_tensor("w_qkvT", [C, 3 * C], XDT, kind="ExternalInput").ap()
    w_oT2 = nc.dram_tensor("w_oT2", [D, H, C], XDT, kind="ExternalInput").ap()
    bn_w = nc.dram_tensor("bn_w", [P, CT, 1], F32, kind="ExternalInput").ap()
    bn_b = nc.dram_tensor("bn_b", [P, CT, 1], F32, kind="ExternalInput").ap()
    b_out = nc.dram_tensor("b_out", [P, RB, 1], F32, kind="ExternalInput").ap()
    out = nc.dram_tensor("out", [C, NH], F32, kind="ExternalOutput").ap()
    with tile.TileContext(nc) as tc:
        _body(tc, x_mine, x_other, x_rest, w_qkvT, w_oT2, bn_w, bn_b, b_out, out)
    nc.compile()
    return nc


_nc_cache = None


def make_in_maps(x, bn_weight, bn_bias, w_qkv, w_out, b_out):
    import ml_dtypes

    x = np.ascontiguousarray(np.asarray(x, dtype=np.float32))
    x_bf = x.astype(ml_dtypes.bfloat16)
    x_f8 = x.astype(ml_dtypes.float8_e4m3fn)
    wqT = np.asarray(w_qkv, dtype=np.float32).T.copy()
    wqT[:, 0:C] *= SCALE  # fold d^-0.5 into the q columns
    wqT = wqT.astype(ml_dtypes.bfloat16)
    # w_out^T reorganized as [d, h, o] for the 4-matmul out-projection
    woT2 = np.ascontiguousarray(
        np.asarray(w_out, dtype=np.float32).T.reshape(H, D, C).transpose(1, 0, 2)
    ).astype(ml_dtypes.bfloat16)

    def vec_layout(v):
        v = np.asarray(v, dtype=np.float32)
        return np.ascontiguousarray(v.reshape(CT, P).T.reshape(P, CT, 1))

    bnw = vec_layout(bn_weight)
    bnb = vec_layout(bn_bias)
    bo = vec_layout(b_out)
    in_maps = []
    # x_rest layout [3, nchunk, P, CT, 1024]: contiguous per DMA chunk so the
    # stats-stream DMAs are pure sequential reads (c = ct*P + p)
    xr_all = x_f8.reshape(B, CT, P, 2, N // 2).transpose(0, 3, 2, 1, 4)
    for core in range(NCORES):
        bi, half = divmod(core, 2)
        mine = np.ascontiguousarray(x_bf[bi][:, half * NH : (half + 1) * NH])
        other = np.ascontiguousarray(x_bf[bi][:, (1 - half) * NH : (2 - half) * NH])
        rest = np.ascontiguousarray(xr_all[[b for b in range(B) if b != bi]])
        in_maps.append(
            {
                "x_mine": mine,
                "x_other": other,
                "x_rest": rest,
                "w_qkvT": wqT,
                "w_oT2": woT2,
                "bn_w": bnw,
                "bn_b": bnb,
                "b_out": bo,
            }
        )
    return in_maps


def assemble(results):
    outp = np.empty((B, C, N), np.float32)
    for core in range(NCORES):
        bi, half = divmod(core, 2)
        outp[bi][:, half * NH : (half + 1) * NH] = results[core]["out"]
    return outp


def kernel(x, bn_weight, bn_bias, w_qkv, w_out, b_out):
    global _nc_cache
    if _nc_cache is None:
        _nc_cache = build()
    in_maps = make_in_maps(x, bn_weight, bn_bias, w_qkv, w_out, b_out)
    res = run_bass_kernel_spmd(_nc_cache, in_maps, list(range(NCORES)))
    return assemble(res.results)


if __name__ == "__main__":
    rng = np.random.default_rng(0)
    x = rng.standard_normal((B, C, N), dtype=np.float32)
    w_qkv = rng.standard_normal((3 * C, C), dtype=np.float32) * C**-0.5
    w_out = rng.standard_normal((C, C), dtype=np.float32) * C**-0.5
    y = kernel(
        x,
        np.ones(C, np.float32),
        np.zeros(C, np.float32),
        w_qkv,
        w_out,
        np.zeros(C, np.float32),
    )
    print(y.shape, np.abs(y).max())


# revision 26
# speedup vs baseline: 1.0088x; 1.0056x over previous
"""TRN2 Bass/Tile kernel: BatchNorm1d + 4-head self-attention + out-projection.

Reference computation (b=4, c=256, n=4096, heads=4, d=64):
    xn   = BN(x)  (training-mode stats over batch+length)
    qkv  = w_qkv @ xn ;  q,k,v  (q scaled by d^-0.5)
    out  = softmax(q^T k) @ v^T  per (batch, head)
    y    = w_out @ out + b_out

Sharding over 8 NeuronCores: core i handles (batch i//2, query-half i%2).
Keys/values are processed in the core-local order [mine, other] (softmax and
attention are invariant to key permutation).

Design notes (v2 rewrite over the AllReduce-free baseline):
  - NO cross-core collective: every core receives the other 3 batches
    (fp8 for stats only) and computes the EXACT global BN statistics
    locally (own batch via DVE bn_stats, part of the rest via ACT
    Copy/Square accum_out sums).
  - BN scale folded into the QKV weights; shift becomes per-channel
    biases.  k-projection bias dropped (per-query score shift, softmax
    cancels it).  v-projection bias folded into the OUTPUT bias:
    W_out @ (W_v @ shift) is a per-output-channel constant, computed
    with tiny PE matmuls at startup -> zero steady-state cost.
  - HEAD-PAIR layout: q/k stored [128, hp, n] with head 2hp on
    partitions 0:64 and head 2hp+1 on 64:128 (no zero padding).  The
    scores for both heads of a pair run as TWO CONCURRENT K=64
    row-tiled matmuls (tile_position (0,0)/(64,0)) - measured 1.88x
    the serial K=128 rate on HW.
  - exp split 50/50: even key-chunks on ACT (table exp), odd chunks on
    the DVE as a Schraudolph bf16 bit-trick (fused mult+add to int16).
  - AV: lhsT = vT-block [128key, 65] bf16 (64 v channels + ones column
    -> softmax denominator for free); attn kept per-head at partitions
    0:64 (attn2 [64, h, n]); out-projection runs as 4 accumulating
    K=64 matmuls per 128-channel block (streaming time only depends on
    the moving size, so this costs ~nothing vs K=128).
  - Normalization (deferred into the next pair, hook chunks 2..13):
    ACT/DVE copy the AV PSUM to SBUF + denominator rows to a
    partition-0 tile (PSUM->partition-0 tensor_copy is the only legal
    cross-partition move), DVE reciprocal_approx_fast, gpsimd
    partition_broadcast, one DVE multiply for both heads.
  - Per chunk the AV matmuls run TWO chunks behind the scores so the
    exp engines (strict ACT/DVE alternation, ACT 17 / DVE 15 chunks)
    are never on the PE's critical path; measured steady chunk period
    873 ns (3x512-col streams + ~200ns exposed LDWEIGHTS - walrus
    emits one LDW per matmul, no dedup, '--enable-ldw-opt=false').
  - PSUM: spool 3x[128,2,512] (scores pairs, outproj bursts, startup
    QKV + keep-warm) + avpool 2x[128,512] (AV accumulators) = 8 banks.
  - BN stats: x_rest arrives as 6 fp8 slabs of 2048 cols on TWO DMA
    rings (sync + gpsimd queues, slab0/slab1 ahead of x in ring
    order); ACT reduces slabs 0,2,3 (Copy/Square accum passes), DVE
    bn_stats slab 1, the own batch in halves, then slabs 4,5.
  - k/v projections are fused INTO the first attention pair (produced
    just-in-time, one key-chunk ahead), so ACT/DVE exp work starts
    ~10us earlier; a data-gated burst of full-array keep-warm matmuls
    (1x1 dummies do NOT register on the HAM activity monitor) runs
    during the stats-combine so QKV hits the 2.4GHz clock; the ACT
    sqrt/exp table sets are preloaded off the critical path.
"""

import numpy as np

import concourse.bacc as bacc
import concourse.tile as tile
from concourse import mybir
from concourse.bass_utils import run_bass_kernel_spmd

B, C, N = 4, 256, 4096
H, D = 4, 64
P = 128
CT = C // P            # 2 channel tiles of 128
RB = 2                 # row blocks for q/k rows (256 = 2*128)
HP = 2                 # head pairs
NH = N // 2            # 2048 queries per core
QS = 512               # query subtile (1 PSUM bank of fp32)
NQS = NH // QS         # 4
KC = 128               # key chunk (matmul stationary width)
NKC = N // KC          # 32
EPS = 1e-5
SCALE = D ** -0.5
F32 = mybir.dt.float32
BF16 = mybir.dt.bfloat16
XDT = BF16
F8 = mybir.dt.float8e4
NCORES = 8
WARM = 10          # keep-warm PE matmuls issued right before stats-combine
SCH_A = 184.6650244    # 2^7 / ln 2
SCH_B = 16250.65       # 127*128 - c_opt (half-way rounding compensation)


def _body(tc, x_mine, x_other, x_rest, w_qkvT, w_oT2, bn_w, bn_b, b_out, out):
    from contextlib import ExitStack

    nc = tc.nc
    AF = mybir.ActivationFunctionType
    OP = mybir.AluOpType

    with ExitStack() as ctx:
        big = ctx.enter_context(tc.tile_pool(name="big", bufs=1))
        small = ctx.enter_context(tc.tile_pool(name="small", bufs=1))
        epool = ctx.enter_context(tc.tile_pool(name="epool", bufs=4))
        oupool = ctx.enter_context(tc.tile_pool(name="oupool", bufs=2))
        rpool = ctx.enter_context(tc.tile_pool(name="rpool", bufs=2))
        opool = ctx.enter_context(tc.tile_pool(name="opool", bufs=2))
        spool = ctx.enter_context(tc.tile_pool(name="spool", bufs=3, space="PSUM"))
        avpool = ctx.enter_context(tc.tile_pool(name="avpool", bufs=2, space="PSUM"))

        # ---- loads (two DMA rings: sync + gpsimd, stats slabs first) ----
        xn_sb = big.tile([P, CT, N], XDT, tag="xn")  # RAW x, key order [mine|other]
        xm_r = x_mine.rearrange("(ct p) n -> p ct n", p=P)
        stg = ctx.enter_context(tc.tile_pool(name="stg", bufs=6))
        NRC_ = 2
        RCW_ = N // NRC_
        sts = [stg.tile([P, CT, RCW_], F8, tag="stg", name=f"st{i}")
               for i in range(6)]
        # sync ring: slab0, x_mine, slab2, slab4, slab5, small vecs
        nc.sync.dma_start(out=sts[0], in_=x_rest[0, 0])
        for ct in range(CT):
            for half in range(2):
                nc.sync.dma_start(
                    out=xn_sb[:, ct, half * (NH // 2) : (half + 1) * (NH // 2)],
                    in_=xm_r[:, ct, half * (NH // 2) : (half + 1) * (NH // 2)],
                )
        nc.sync.dma_start(out=sts[2], in_=x_rest[1, 0])
        nc.sync.dma_start(out=sts[4], in_=x_rest[2, 0])
        nc.sync.dma_start(out=sts[5], in_=x_rest[2, 1])
        bnw_sb = small.tile([P, CT, 1], F32)
        nc.sync.dma_start(out=bnw_sb, in_=bn_w)
        bnb_sb = small.tile([P, CT, 1], F32)
        nc.sync.dma_start(out=bnb_sb, in_=bn_b)
        bo_sb = small.tile([P, RB, 1], F32)
        nc.sync.dma_start(out=bo_sb, in_=b_out)
        # gpsimd ring: slab1, x_other, slab3, weights (x_other early: it
        # gates the own-batch second-half bn_stats on the DVE)
        nc.gpsimd.dma_start(out=sts[1], in_=x_rest[0, 1])
        nc.gpsimd.dma_start(
            out=xn_sb[:, :, NH:N], in_=x_other.rearrange("(ct p) n -> p ct n", p=P)
        )
        nc.gpsimd.dma_start(out=sts[3], in_=x_rest[1, 1])
        wq_sb = big.tile([P, CT, 3 * C], XDT)
        nc.gpsimd.dma_start(
            out=wq_sb, in_=w_qkvT.rearrange("(ct p) o -> p ct o", p=P)
        )
        wo2_sb = big.tile([D, H, C], XDT)   # w_out^T as [d, h, o]
        nc.gpsimd.dma_start(out=wo2_sb, in_=w_oT2)

        # ---- ACT table preloads -----------------------------------------
        # table sets are exclusive; load the sqrt set early (stats' Copy/
        # Square ride along in any set) so the combine's Sqrt does not pay
        # the ~1.3us ACT_TABLE_LOAD on the critical path.
        tiny = small.tile([1, 2], F32)
        nc.scalar.activation(out=tiny[:, 0:1], in_=bnw_sb[0:1, 0, :],
                             func=AF.Sqrt)

        # ---- BN stats: EXACT global stats computed locally --------------
        # ACT: Sum/Sum^2 accum passes over slabs 0,2,3 (earliest arrivals);
        # DVE: bn_stats over slab1 (arrives before x_mine completes), the
        # own batch in halves (mine half first), then slabs 4,5.
        RCW = RCW_
        SG = N // 512      # own-batch 512-col stat groups per ct
        ACT_SL = (0, 3, 2)
        DVE_SL = (1, 4, 5)
        NACT = len(ACT_SL)
        RG = RCW // 512
        NDVE_R = len(DVE_SL) * RG
        NREC = SG + NDVE_R         # bn_stats records per ct
        NS = N + NDVE_R * 512      # samples covered by bn_stats records
        NT = B * N                 # total samples per channel
        stat6 = small.tile([P, CT, NREC, 6], F32)
        trash = small.tile([P, RCW], BF16)
        acc_x = small.tile([P, CT, NACT], F32)
        acc_x2 = small.tile([P, CT, NACT], F32)

        def dve_slab(si, ri):
            st = sts[si]
            for ct in range(CT):
                xr = st[:, ct, :].rearrange("p (s f) -> p s f", f=512)
                for g in range(RG):
                    nc.vector.bn_stats(
                        out=stat6[:, ct, SG + ri * RG + g, :], in_=xr[:, g, :]
                    )

        def own_half(half):
            for ct in range(CT):
                xm = xn_sb[:, ct, :].rearrange("p (s f) -> p s f", f=512)
                for g in range(half * 4, half * 4 + 4):
                    nc.vector.bn_stats(out=stat6[:, ct, g, :], in_=xm[:, g, :])

        dve_slab(DVE_SL[0], 0)
        own_half(0)
        own_half(1)
        dve_slab(DVE_SL[1], 1)
        dve_slab(DVE_SL[2], 2)
        for ci, si in enumerate(ACT_SL):
            for ct in range(CT):
                nc.scalar.activation(
                    out=trash, in_=sts[si][:, ct, :], func=AF.Copy,
                    accum_out=acc_x[:, ct, ci : ci + 1],
                )
                nc.scalar.activation(
                    out=trash, in_=sts[si][:, ct, :], func=AF.Square,
                    accum_out=acc_x2[:, ct, ci : ci + 1],
                )

        # ---- PE keep-warm ------------------------------------------------
        # The HAM activity monitor runs the PE at half clock until it has
        # seen a ~3.4us busy window.  A dense burst of N=512 matmuls DATA-
        # GATED on stat6 (so the scheduler cannot hoist it) runs during the
        # stats-combine and warms the clock just in time for the QKV
        # projections.
        dum2 = small.tile([P, QS], BF16)
        nc.vector.memset(dum2, 0.001)
        nc.vector.tensor_copy(
            out=dum2[0:1, 0 : NREC * 6],
            in_=stat6[0:1, 0, :, :].rearrange("p a b -> p (a b)"),
        )
        for i in range(WARM):
            scrap = spool.tile([P, 2, QS], F32, tag="sp", name="scrap")
            nc.tensor.matmul(
                out=scrap[:, 0, :], lhsT=dum2[:, 0:P], rhs=dum2,
                start=True, stop=True,
            )

        mv = small.tile([P, CT, 2], F32)
        for ct in range(CT):
            nc.vector.bn_aggr(out=mv[:, ct, :], in_=stat6[:, ct])
        # combine: totals = bn_aggr subset (NS samples) + ACT sums
        sum_t = small.tile([P, CT, 1], F32)
        nc.vector.tensor_reduce(
            out=sum_t, in_=acc_x, axis=mybir.AxisListType.X,
            op=mybir.AluOpType.add,
        )
        sq_t = small.tile([P, CT, 1], F32)
        nc.vector.tensor_reduce(
            out=sq_t, in_=acc_x2, axis=mybir.AxisListType.X,
            op=mybir.AluOpType.add,
        )
        msq_s = small.tile([P, CT, 1], F32)
        nc.vector.tensor_mul(out=msq_s, in0=mv[:, :, 0:1], in1=mv[:, :, 0:1])
        e2_s = small.tile([P, CT, 1], F32)
        nc.vector.tensor_add(out=e2_s, in0=mv[:, :, 1:2], in1=msq_s)
        # sum_t += mean_s * NS ; sq_t += e2_s * NS
        tmp_s = small.tile([P, CT, 1], F32)
        nc.vector.tensor_scalar_mul(out=tmp_s, in0=mv[:, :, 0:1], scalar1=float(NS))
        nc.vector.tensor_add(out=sum_t, in0=sum_t, in1=tmp_s)
        nc.vector.tensor_scalar_mul(out=tmp_s, in0=e2_s, scalar1=float(NS))
        nc.vector.tensor_add(out=sq_t, in0=sq_t, in1=tmp_s)
        mvg = small.tile([P, CT, 2], F32)
        nc.vector.tensor_scalar_mul(
            out=mvg[:, :, 0:1], in0=sum_t, scalar1=1.0 / NT
        )
        nc.vector.tensor_scalar_mul(out=tmp_s, in0=sq_t, scalar1=1.0 / NT)
        nc.vector.tensor_mul(
            out=mvg[:, :, 1:2], in0=mvg[:, :, 0:1], in1=mvg[:, :, 0:1]
        )
        nc.vector.tensor_sub(out=mvg[:, :, 1:2], in0=tmp_s, in1=mvg[:, :, 1:2])
        mv = mvg

        eps_sb = small.tile([P, 1], F32)
        nc.vector.memset(eps_sb, EPS)

        # ---- global mean/var -> s = bn_w * rstd, shift = bn_b - mean*s --
        mean_g = mv[:, :, 0:1]
        var_g = mv[:, :, 1:2]
        sd = small.tile([P, CT, 1], F32)
        nc.scalar.activation(out=sd, in_=var_g, func=AF.Sqrt, bias=eps_sb)
        nc.scalar.activation(out=tiny[:, 1:2], in_=sd[0:1, 0, :], func=AF.Exp)
        rstd = small.tile([P, CT, 1], F32)
        nc.vector.reciprocal(out=rstd, in_=sd)
        s_sb = small.tile([P, CT, 1], F32)
        nc.vector.tensor_mul(out=s_sb, in0=bnw_sb, in1=rstd)
        shift_sb = small.tile([P, CT, 1], F32)
        nc.vector.tensor_mul(out=shift_sb, in0=mean_g, in1=s_sb)
        nc.vector.tensor_sub(out=shift_sb, in0=bnb_sb, in1=shift_sb)
        shift_bf = small.tile([P, CT, 1], BF16)
        nc.vector.tensor_copy(out=shift_bf, in_=shift_sb)

        # ---- biases from the ORIGINAL weights ---------------------------
        # q bias qkb[:, rb] = W_q[rb] @ shift (k bias dropped: softmax
        # cancels a per-query score shift).
        # v bias per head as a [64,1] column: vb4[:, h] = W_v,h @ shift.
        # Output-bias correction: bo_eff = b_out + W_out @ vb  (the v bias
        # contributes attn-weight-sum * vb = vb after normalization).
        bias_ps = spool.tile([P, 2, QS], F32, tag="sp", name="bias")
        for rb in range(RB):
            for ct in range(CT):
                nc.tensor.matmul(
                    out=bias_ps[:, 0, rb : rb + 1],
                    lhsT=wq_sb[:, ct, rb * P : (rb + 1) * P],
                    rhs=shift_bf[:, ct],
                    start=(ct == 0),
                    stop=(ct == CT - 1),
                )
        for h in range(H):
            for ct in range(CT):
                nc.tensor.matmul(
                    out=bias_ps[0:D, 0, RB + h : RB + h + 1],
                    lhsT=wq_sb[:, ct, 2 * C + h * D : 2 * C + (h + 1) * D],
                    rhs=shift_bf[:, ct],
                    start=(ct == 0),
                    stop=(ct == CT - 1),
                )
        qkb_sb = small.tile([P, RB], F32)
        nc.vector.tensor_copy(out=qkb_sb, in_=bias_ps[:, 0, 0:RB])
        vb4 = small.tile([D, H], BF16)
        nc.vector.tensor_copy(out=vb4, in_=bias_ps[0:D, 0, RB : RB + H])
        # bo_eff = b_out + sum_h W_out[:, (h,:)] @ vb4[:, h]
        for rb in range(RB):
            for h in range(H):
                nc.tensor.matmul(
                    out=bias_ps[:, 1, rb : rb + 1],
                    lhsT=wo2_sb[:, h, rb * P : (rb + 1) * P],
                    rhs=vb4[:, h : h + 1],
                    start=(h == 0),
                    stop=(h == H - 1),
                )
        bo_eff = small.tile([P, RB], F32)
        nc.vector.tensor_add(
            out=bo_eff, in0=bias_ps[:, 1, 0:RB],
            in1=bo_sb.rearrange("p rb one -> p (rb one)"),
        )

        # fold diag(s) into the weights, q columns first so q matmuls can
        # start while k/v columns are still being scaled (d^-0.5 is folded
        # into the q columns host-side)
        for sec in range(3):
            for ct in range(CT):
                nc.vector.tensor_scalar_mul(
                    out=wq_sb[:, ct, sec * C : (sec + 1) * C],
                    in0=wq_sb[:, ct, sec * C : (sec + 1) * C],
                    scalar1=s_sb[:, ct],
                )

        # ---- attention state --------------------------------------------
        q2 = big.tile([P, HP, NH], BF16)
        k2 = big.tile([P, HP, N], BF16)
        vT = big.tile([P, NKC, H, D + 1], BF16)
        nc.gpsimd.memset(vT[:, :, :, D : D + 1], 1.0)
        attn2 = big.tile([D, H, NH], BF16)
        out_r = out.rearrange("(rb p) n -> p rb n", p=P)

        # ---- q projection (consumes RAW x; bias fused into copies) ------
        for j in range(NQS):
            ps = spool.tile([P, 2, QS], F32, tag="sp", name="qproj")
            for hp in range(HP):
                for ct in range(CT):
                    nc.tensor.matmul(
                        out=ps[:, hp, :],
                        lhsT=wq_sb[:, ct, hp * P : (hp + 1) * P],
                        rhs=xn_sb[:, ct, j * QS : (j + 1) * QS],
                        start=(ct == 0),
                        stop=(ct == CT - 1),
                    )
            nc.vector.tensor_scalar_add(
                out=q2[:, 0, j * QS : (j + 1) * QS], in0=ps[:, 0, :],
                scalar1=qkb_sb[:, 0:1],
            )
            nc.scalar.activation(
                out=q2[:, 1, j * QS : (j + 1) * QS], in_=ps[:, 1, :],
                func=AF.Identity, bias=qkb_sb[:, 1:2],
            )

        # ---- k/v production (fused into the first pair's chunk loop) ---
        def emit_kproj(ps_slot, hp, jk, eng):
            for ct in range(CT):
                nc.tensor.matmul(
                    out=ps_slot,
                    lhsT=wq_sb[:, ct, C + hp * P : C + (hp + 1) * P],
                    rhs=xn_sb[:, ct, jk * QS : (jk + 1) * QS],
                    start=(ct == 0),
                    stop=(ct == CT - 1),
                )
            dst = k2[:, hp, jk * QS : (jk + 1) * QS]
            if eng == "act":
                nc.scalar.activation(out=dst, in_=ps_slot, func=AF.Identity)
            else:
                nc.vector.tensor_copy(out=dst, in_=ps_slot)

        def emit_vproj(ps_slot, nb):
            for ct in range(CT):
                nc.tensor.matmul(
                    out=ps_slot[:, 0:C],
                    lhsT=xn_sb[:, ct, nb * KC : (nb + 1) * KC],
                    rhs=wq_sb[:, ct, 2 * C : 3 * C],
                    start=(ct == 0),
                    stop=(ct == CT - 1),
                )
            nc.vector.tensor_copy(
                out=vT[:, nb, :, 0:D],
                in_=ps_slot[:, 0:C].rearrange("p (h d) -> p h d", d=D),
            )

        # ---- attention ---------------------------------------------------
        # Normalize/outproj of pair X are deferred into pair X+1 (hooks at
        # fixed chunk indices) so they never stall the exp engines.  The AV
        # matmuls run one chunk behind the scores (pend_av), carried across
        # pair boundaries.
        pend_av = []          # queue of up to 2 pending AV chunk emissions
        pend_norm = [None]

        def emit_av(p, us=(0, 1)):
            e_, c_, avp_, hp_ = p
            for u in us:
                nc.tensor.matmul(
                    out=avp_[u][0 : D + 1, :],
                    lhsT=vT[:, c_, 2 * hp_ + u, :],
                    rhs=e_[:, u, :],
                    start=(c_ == 0),
                    stop=(c_ == NKC - 1),
                )

        def outproj_mm(ps, j_, step):
            rb, h = divmod(step, H)
            nc.tensor.matmul(
                out=ps[:, rb, :],
                lhsT=wo2_sb[:, h, rb * P : (rb + 1) * P],
                rhs=attn2[:, h, j_ * QS : (j_ + 1) * QS],
                start=(h == 0),
                stop=(h == H - 1),
            )

        def outproj_fin(ps, j_):
            o_t = opool.tile([P, RB, QS], F32, tag="o")
            for rb in range(RB):
                nc.scalar.activation(
                    out=o_t[:, rb, :], in_=ps[:, rb, :],
                    func=AF.Identity, bias=bo_eff[:, rb : rb + 1],
                )
            nc.sync.dma_start(
                out=out_r[:, :, j_ * QS : (j_ + 1) * QS], in_=o_t
            )

        def outproj(j_):
            ps = spool.tile([P, 2, QS], F32, tag="sp", name="oproj")
            for step in range(2 * H):
                outproj_mm(ps, j_, step)
            outproj_fin(ps, j_)

        def sch_exp(e_sb, sp):
            # Schraudolph bf16 exp on the DVE: one fused mult+add into
            # int16, bitcast to bf16 (~1.5% rms err)
            nc.vector.tensor_scalar(
                out=e_sb.bitcast(mybir.dt.int16),
                in0=sp,
                scalar1=SCH_A,
                scalar2=SCH_B,
                op0=OP.mult,
                op1=OP.add,
            )

        def pair(j, hp, fused):
            prev = pend_norm[0]
            ou_ref = [None]
            rbc_ref = [None]
            avp = (
                avpool.tile([P, QS], F32, tag="av", name="avp0"),
                avpool.tile([P, QS], F32, tag="av", name="avp1"),
            )
            vt_ref = [None]
            op_ref = [None]
            for c in range(NKC):
                if fused:
                    jk = c // 4
                    if c % 4 == 0:
                        kt = spool.tile([P, 2, QS], F32, tag="sp", name="kproj")
                        emit_kproj(kt[:, 0, :], 0, jk, "act")
                        emit_kproj(kt[:, 1, :], 1, jk, "dve")
                    if c % 2 == 0:
                        vt_ref[0] = spool.tile(
                            [P, 2, QS], F32, tag="sp", name="vproj"
                        )
                    emit_vproj(vt_ref[0][:, c % 2, :], c)
                sp = spool.tile([P, 2, QS], F32, tag="sp", name="sp")
                nc.tensor.matmul(
                    out=sp[:, 0, :],
                    lhsT=k2[0:D, hp, c * KC : (c + 1) * KC],
                    rhs=q2[0:D, hp, j * QS : (j + 1) * QS],
                    start=True, stop=True,
                )
                nc.tensor.matmul(
                    out=sp[:, 1, :],
                    lhsT=k2[D:P, hp, c * KC : (c + 1) * KC],
                    rhs=q2[D:P, hp, j * QS : (j + 1) * QS],
                    start=True, stop=True,
                )
                e_sb = epool.tile([P, 2, QS], BF16, tag="e")
                if fused:
                    if c % 3 == 2:
                        sch_exp(e_sb, sp)
                    else:
                        nc.scalar.activation(out=e_sb, in_=sp, func=AF.Exp)
                elif c % 2 == 0 and c > 0:
                    sch_exp(e_sb, sp)
                else:
                    nc.scalar.activation(out=e_sb, in_=sp, func=AF.Exp)
                # deferred normalize of the previous pair.  ALL reads of
                # pavp[u] must be emitted at c == 1+u (before the new AV
                # matmul to that PSUM bank is emitted).
                if prev is not None:
                    pj, php, pavp = prev
                    if c == 2:
                        ou = oupool.tile([D, 2, QS], F32, tag="ou")
                        ou_ref[0] = ou
                        den = oupool.tile([1, 2, QS], F32, tag="den")
                        rbc_ref[0] = [den, None]
                        nc.scalar.activation(
                            out=ou[:, 0, :], in_=pavp[0][0:D, :],
                            func=AF.Identity,
                        )
                        nc.vector.tensor_copy(
                            out=den[:, 0, :], in_=pavp[0][D : D + 1, :]
                        )
                    elif c == 3:
                        nc.vector.tensor_copy(
                            out=ou_ref[0][:, 1, :], in_=pavp[1][0:D, :]
                        )
                        nc.vector.tensor_copy(
                            out=rbc_ref[0][0][:, 1, :], in_=pavp[1][D : D + 1, :]
                        )
                    elif c == 5:
                        r0 = oupool.tile([1, 2, QS], F32, tag="r0")
                        nc.vector.reciprocal_approx_fast(
                            out=r0, in_=rbc_ref[0][0]
                        )
                        rbc_ref[0][0] = r0
                    elif c == 7:
                        rbc = rpool.tile([D, 2, QS], F32, tag="rbc")
                        rbc_ref[0][1] = rbc
                        for u in range(2):
                            nc.gpsimd.partition_broadcast(
                                rbc[:, u, :], rbc_ref[0][0][0:1, u, :]
                            )
                    elif c == 10:
                        nc.vector.tensor_tensor(
                            out=attn2[:, 2 * php : 2 * php + 2,
                                      pj * QS : (pj + 1) * QS],
                            in0=ou_ref[0],
                            in1=rbc_ref[0][1],
                            op=OP.mult,
                        )
                    elif c == 13 and php == 1:
                        outproj(pj)
                if len(pend_av) >= 2:
                    emit_av(pend_av.pop(0))
                pend_av.append((e_sb, c, avp, hp))
            pend_norm[0] = (j, hp, avp)

        first = True
        for j in range(NQS):
            for hp in range(HP):
                pair(j, hp, fused=first)
                first = False
        while pend_av:
            emit_av(pend_av.pop(0))

        # flush the last pair's normalize + outproj inline.  A few dummy
        # matmuls keep the HAM clock warm through the serial normalize
        # chain so the final out-projection runs at full rate.
        for i in range(8):
            scrap = spool.tile([P, 2, QS], F32, tag="sp", name="scrap2")
            nc.tensor.matmul(
                out=scrap[:, 0, :], lhsT=dum2[:, 0:P], rhs=dum2,
                start=True, stop=True,
            )
        pj, php, pavp = pend_norm[0]
        ou = oupool.tile([D, 2, QS], F32, tag="ou")
        den = oupool.tile([1, 2, QS], F32, tag="den")
        nc.scalar.activation(out=ou[:, 0, :], in_=pavp[0][0:D, :],
                             func=AF.Identity)
        nc.vector.tensor_copy(out=den[:, 0, :], in_=pavp[0][D : D + 1, :])
        nc.vector.tensor_copy(out=ou[:, 1, :], in_=pavp[1][0:D, :])
        nc.vector.tensor_copy(out=den[:, 1, :], in_=pavp[1][D : D + 1, :])
        r0 = oupool.tile([1, 2, QS], F32, tag="r0")
        nc.vector.reciprocal_approx_fast(out=r0, in_=den)
        rbc = rpool.tile([D, 2, QS], F32, tag="rbc")
        nc.gpsimd.partition_broadcast(
            rbc.rearrange("p a b -> p (a b)"),
            r0[0:1, :, :].rearrange("p a b -> p (a b)"),
        )
        nc.vector.tensor_tensor(
            out=attn2[:, 2 * php : 2 * php + 2, pj * QS : (pj + 1) * QS],
            in0=ou, in1=rbc, op=OP.mult,
        )
        outproj(pj)


def build():
    nc = bacc.Bacc(
        "TRN2", target_bir_lowering=False, debug=False, num_devices=NCORES
    )
    x_mine = nc.dram_tensor("x_mine", [C, NH], XDT, kind="ExternalInput").ap()
    x_other = nc.dram_tensor("x_other", [C, NH], XDT, kind="ExternalInput").ap()
    x_rest = nc.dram_tensor(
        "x_rest", [3, 2, P, CT, N // 2], F8, kind="ExternalInput"
    ).ap()
    w_qkvT = nc.dram_tensor("w_qkvT", [C, 3 * C], XDT, kind="ExternalInput").ap()
    w_oT2 = nc.dram_tensor("w_oT2", [D, H, C], XDT, kind="ExternalInput").ap()
    bn_w = nc.dram_tensor("bn_w", [P, CT, 1], F32, kind="ExternalInput").ap()
    bn_b = nc.dram_tensor("bn_b", [P, CT, 1], F32, kind="ExternalInput").ap()
    b_out = nc.dram_tensor("b_out", [P, RB, 1], F32, kind="ExternalInput").ap()
    out = nc.dram_tensor("out", [C, NH], F32, kind="ExternalOutput").ap()
    with tile.TileContext(nc) as tc:
        _body(tc, x_mine, x_other, x_rest, w_qkvT, w_oT2, bn_w, bn_b, b_out, out)
    nc.compile()
    return nc


_nc_cache = None


def make_in_maps(x, bn_weight, bn_bias, w_qkv, w_out, b_out):
    import ml_dtypes

    x = np.ascontiguousarray(np.asarray(x, dtype=np.float32))
    x_bf = x.astype(ml_dtypes.bfloat16)
    x_f8 = x.astype(ml_dtypes.float8_e4m3fn)
    wqT = np.asarray(w_qkv, dtype=np.float32).T.copy()
    wqT[:, 0:C] *= SCALE  # fold d^-0.5 into the q columns
    wqT = wqT.astype(ml_dtypes.bfloat16)
    # w_out^T reorganized as [d, h, o] for the 4-matmul out-projection
    woT2 = np.ascontiguousarray(
        np.asarray(w_out, dtype=np.float32).T.reshape(H, D, C).transpose(1, 0, 2)
    ).astype(ml_dtypes.bfloat16)

    def vec_layout(v):
        v = np.asarray(v, dtype=np.float32)
        return np.ascontiguousarray(v.reshape(CT, P).T.reshape(P, CT, 1))

    bnw = vec_layout(bn_weight)
    bnb = vec_layout(bn_bias)
    bo = vec_layout(b_out)
    in_maps = []
    # x_rest layout [3, nchunk, P, CT, 1024]: contiguous per DMA chunk so the
    # stats-stream DMAs are pure sequential reads (c = ct*P + p)
    xr_all = x_f8.reshape(B, CT, P, 2, N // 2).transpose(0, 3, 2, 1, 4)
    for core in range(NCORES):
        bi, half = divmod(core, 2)
        mine = np.ascontiguousarray(x_bf[bi][:, half * NH : (half + 1) * NH])
        other = np.ascontiguousarray(x_bf[bi][:, (1 - half) * NH : (2 - half) * NH])
        rest = np.ascontiguousarray(xr_all[[b for b in range(B) if b != bi]])
        in_maps.append(
            {
                "x_mine": mine,
                "x_other": other,
                "x_rest": rest,
                "w_qkvT": wqT,
                "w_oT2": woT2,
                "bn_w": bnw,
                "bn_b": bnb,
                "b_out": bo,
            }
        )
    return in_maps


def assemble(results):
    outp = np.empty((B, C, N), np.float32)
    for core in range(NCORES):
        bi, half = divmod(core, 2)
        outp[bi][:, half * NH : (half + 1) * NH] = results[core]["out"]
    return outp


def kernel(x, bn_weight, bn_bias, w_qkv, w_out, b_out):
    global _nc_cache
    if _nc_cache is None:
        _nc_cache = build()
    in_maps = make_in_maps(x, bn_weight, bn_bias, w_qkv, w_out, b_out)
    res = run_bass_kernel_spmd(_nc_cache, in_maps, list(range(NCORES)))
    return assemble(res.results)


if __name__ == "__main__":
    rng = np.random.default_rng(0)
    x = rng.standard_normal((B, C, N), dtype=np.float32)
    w_qkv = rng.standard_normal((3 * C, C), dtype=np.float32) * C**-0.5
    w_out = rng.standard_normal((C, C), dtype=np.float32) * C**-0.5
    y = kernel(
        x,
        np.ones(C, np.float32),
        np.zeros(C, np.float32),
        w_qkv,
        w_out,
        np.zeros(C, np.float32),
    )
    print(y.shape, np.abs(y).max())
